# revision 2
# baseline (speedup 1.0000x reference)
"""Trainium2 Bass kernel for nn_AttentionLayer (B=64, S=512, F=256), 8 cores.

Reference computation (per batch b):
    scores = x1 @ Wq + x2 @ Wk          # [S, S]
    a = softmax(tanh(scores), axis=-1)   # softmax over u
    a2 = a @ Wv                          # [S, S]
    out = a2 * x1                        # elementwise
    out = out * rsqrt(max(sum_s out^2, eps))   # l2-normalize over axis s

Strategy: pure data parallelism -- 8 batches per core, weights replicated.
Everything is computed in a TRANSPOSED layout ([t-or-u partitions, s free]).

v2 changes vs the f32r baseline:
  * Stage-A matmuls run in SPLIT-fp8 DoubleRow mode: x1/x2 and Wq/Wk are
    decomposed on the host into hi+lo float8_e4m3 digits (W scaled by 64);
    the scores are formed from the hh+hl+lh digit products as fp8 DoubleRow
    matmuls (2 contraction k-tiles per instruction at 0.5 cycles/row), which
    is 25% fewer PE cycles than f32r at bf16-level accuracy.
  * The softmax is kept UNNORMALIZED through stage C (y = Wv^T @ expz); the
    1/denominator is folded into the epilogue elementwise stage, so no PE
    work ever waits on the rowsum->recip->broadcast chain.
  * Epilogue is bf16 on DVE (2x mode where operands allow): q = y*x1,
    w = q*recip_bcast, sum-of-squares via stt-accumulate; the final
    per-partition 1/sqrt scale runs on GpSimd as tensor_scalar multiply.
  * All DRAM tensors are packed partition-major on the host so each batch
    needs only one DMA call per tensor with >=2KB contiguous lines.
  * Output is produced transposed in bf16 and untransposed/upcast on host.
"""

import sys

sys.path.insert(0, "/opt/trn_rl_repo")

import numpy as np
import ml_dtypes

import concourse.bass as bass
import concourse.tile as tile
from concourse import bacc, mybir
from concourse.bass_utils import run_bass_kernel_spmd

B, S, F = 64, 512, 256
N_CORES = 8
BPC = B // N_CORES  # batches per core
P = 128
KT1 = S // P  # 4 k-tiles over t (x1/Wq contraction)
KT2 = F // P  # 2 k-tiles over f (x2/Wk contraction)
NT = S // P  # 4 m-tiles over u (stage A) / t (stage C)
EPS = 1e-12
WSCALE = 64.0  # weights are scaled by this before fp8 split; tanh scale undoes it

F32 = mybir.dt.float32
BF16 = mybir.dt.bfloat16
F8 = mybir.dt.float8e4
AF = mybir.ActivationFunctionType
ALU = mybir.AluOpType
DR = mybir.MatmulPerfMode.DoubleRow

E4NP = ml_dtypes.float8_e4m3  # numpy dtype matching mybir float8e4
BFNP = ml_dtypes.bfloat16

last_results = None  # test harness introspection


def build_nc(reps=1, bpc=BPC):
    nc = bacc.Bacc(
        "TRN2", target_bir_lowering=False, debug=False, num_devices=N_CORES
    )
    # Partition-major packed inputs: [.., P, chunks, S] so one DMA per tensor.
    # x1hl chunks: 0-3 = hi digit k-tiles, 4-7 = lo digit k-tiles (t dim).
    x1hl = nc.declare_dram_parameter("x1hl", [bpc, P, 2 * KT1, S], F8, isOutput=False)
    x2hl = nc.declare_dram_parameter("x2hl", [bpc, P, 2 * KT2, S], F8, isOutput=False)
    x1b = nc.declare_dram_parameter("x1b", [bpc, P, KT1, S], BF16, isOutput=False)
    wq = nc.declare_dram_parameter("wq", [P, 2 * KT1, S], F8, isOutput=False)
    wk = nc.declare_dram_parameter("wk", [P, 2 * KT2, S], F8, isOutput=False)
    wv = nc.declare_dram_parameter("wv", [P, NT, S], BF16, isOutput=False)
    out = nc.declare_dram_parameter("out", [bpc, P, NT, S], BF16, isOutput=True)

    batches = [bb for _ in range(reps) for bb in range(bpc)]

    with tile.TileContext(nc) as tc:
        with (
            tc.tile_pool(name="singles", bufs=1) as singles,
            tc.tile_pool(name="xin", bufs=1) as xin,
            tc.tile_pool(name="work", bufs=2) as work,
            tc.tile_pool(name="small", bufs=2) as small,
            tc.tile_pool(name="outp", bufs=2) as outp,
            tc.tile_pool(name="psA", bufs=2, space="PSUM") as psA,
            tc.tile_pool(name="psY", bufs=3, space="PSUM") as psY,
            tc.tile_pool(name="psR", bufs=1, space="PSUM") as psR,
        ):
            # Startup DMAs: first batch's x1 + weights round-robin across
            # queues so the first matmuls aren't serialized on one queue.
            b0 = batches[0]
            x1_first = xin.tile([P, 2 * KT1, S], F8, tag="x1", bufs=3)
            nc.sync.dma_start(out=x1_first, in_=x1hl.ap()[b0])
            wq_t = singles.tile([P, 2 * KT1, S], F8, tag="wq")
            engs = [nc.scalar, nc.gpsimd, nc.scalar, nc.gpsimd]
            for c in range(4):
                engs[c].dma_start(
                    out=wq_t[:, 2 * c : 2 * c + 2, :], in_=wq.ap()[:, 2 * c : 2 * c + 2, :]
                )
            wk_t = singles.tile([P, 2 * KT2, S], F8, tag="wk")
            nc.scalar.dma_start(out=wk_t, in_=wk.ap())
            x2_first = xin.tile([P, 2 * KT2, S], F8, tag="x2", bufs=3)
            nc.sync.dma_start(out=x2_first, in_=x2hl.ap()[b0])
            wv_t = singles.tile([P, NT, S], BF16, tag="wv")
            nc.gpsimd.dma_start(out=wv_t, in_=wv.ap())
            x1b_first = xin.tile([P, KT1, S], BF16, tag="x1b", bufs=3)
            nc.gpsimd.dma_start(out=x1b_first, in_=x1b.ap()[b0])

            ones_col = singles.tile([P, 1], BF16)
            nc.vector.memset(ones_col, 1.0)
            eps_t = singles.tile([P, 1], F32)
            nc.vector.memset(eps_t, EPS)

            def stage_a(b, x1_sb, x2_sb, mid_cb=None):
                """36 fp8 DoubleRow matmuls (hh+hl+lh digit products) into
                u-tile pair PSUM tiles, then tanh(psum/WSCALE) + exp on ACT.
                mid_cb (if set) is emitted between the two halves so the
                previous batch's rowsum overlaps this batch's matmuls."""
                expz = work.tile([P, NT, S], BF16, tag="expz", bufs=3)
                for half in range(NT // 2):
                    sc = psA.tile([P, 2, S], F32, tag="scores")
                    for j in range(2):
                        ut = half * 2 + j
                        us = slice(ut * P, (ut + 1) * P)
                        # (weight digit slice, moving digit slice) pairs;
                        # hi k-pairs of x1 are chunks 0:2/2:4, lo 4:6/6:8.
                        seq = [
                            (wq_t[:, 0:2, us], x1_sb[:, 0:2, :]),  # hh pair 0
                            (wq_t[:, 2:4, us], x1_sb[:, 2:4, :]),  # hh pair 1
                            (wq_t[:, 4:6, us], x1_sb[:, 0:2, :]),  # lh pair 0
                            (wq_t[:, 6:8, us], x1_sb[:, 2:4, :]),  # lh pair 1
                            (wq_t[:, 0:2, us], x1_sb[:, 4:6, :]),  # hl pair 0
                            (wq_t[:, 2:4, us], x1_sb[:, 6:8, :]),  # hl pair 1
                            (wk_t[:, 0:2, us], x2_sb[:, 0:2, :]),  # hh
                            (wk_t[:, 2:4, us], x2_sb[:, 0:2, :]),  # lh
                            (wk_t[:, 0:2, us], x2_sb[:, 2:4, :]),  # hl
                        ]
                        for i, (l_ap, r_ap) in enumerate(seq):
                            nc.tensor.matmul(
                                sc[:, j, :],
                                l_ap,
                                r_ap,
                                start=(i == 0),
                                stop=(i == len(seq) - 1),
                                perf_mode=DR,
                            )
                    tanh_t = work.tile([P, 2, S], F32, tag="tanh")
                    nc.scalar.activation(
                        out=tanh_t, in_=sc, func=AF.Tanh, scale=1.0 / WSCALE
                    )
                    nc.scalar.activation(
                        out=expz[:, half * 2 : half * 2 + 2, :],
                        in_=tanh_t,
                        func=AF.Exp,
                    )
                    if half == 0 and mid_cb is not None:
                        mid_cb()
                return expz

            def stage_b(b, expz):
                """softmax denominator: rowsum matmul -> recip -> broadcast."""
                rs = psR.tile([1, S], F32, tag="rowsum")
                for ut in range(NT):
                    nc.tensor.matmul(
                        rs,
                        ones_col,
                        expz[:, ut, :],
                        start=(ut == 0),
                        stop=(ut == NT - 1),
                    )
                recip_f = small.tile([1, S], F32, tag="recipf")
                nc.vector.reciprocal_approx_fast(out=recip_f, in_=rs)
                recip = small.tile([1, S], BF16, tag="recip")
                nc.vector.tensor_copy(out=recip, in_=recip_f)
                rbc = small.tile([P, S], BF16, tag="rbc")
                nc.gpsimd.partition_broadcast(rbc, recip)
                return rbc

            def stage_c(b, x1b_sb, expz, rbc):
                """Y matmuls on raw expz; epilogue q=y*x1, w=q*rbc, sumsq."""
                w_sb = outp.tile([P, NT, S], BF16, tag="w")
                sumsq = small.tile([P, NT], F32, tag="sumsq", bufs=4)
                for tt in range(NT):
                    y = psY.tile([P, S], F32, tag="y")
                    for ut in range(NT):
                        nc.tensor.matmul(
                            y,
                            wv_t[:, ut, tt * P : (tt + 1) * P],
                            expz[:, ut, :],
                            start=(ut == 0),
                            stop=(ut == NT - 1),
                        )
                    q_t = small.tile([P, S], BF16, tag="q")
                    nc.vector.scalar_tensor_tensor(
                        out=q_t,
                        in0=y,
                        scalar=1.0,
                        in1=x1b_sb[:, tt, :],
                        op0=ALU.mult,
                        op1=ALU.mult,
                    )
                    w_t = w_sb[:, tt, :]
                    nc.vector.tensor_tensor(out=w_t, in0=q_t, in1=rbc, op=ALU.mult)
                    scr = small.tile([P, S], BF16, tag="scr")
                    nc.vector.scalar_tensor_tensor(
                        out=scr,
                        in0=w_t,
                        scalar=1.0,
                        in1=w_t,
                        op0=ALU.mult,
                        op1=ALU.mult,
                        accum_out=sumsq[:, tt : tt + 1],
                    )
                return w_sb, sumsq

            def stage_fin(b, w_sb, sumsq):
                """sqrt (ACT, emitted adjacently for pairs of batches to halve
                activation-table swaps), recip, GpSimd scale, store bf16."""
                rsq = small.tile([P, NT], F32, tag="rsq", bufs=4)
                nc.scalar.activation(out=rsq, in_=sumsq, func=AF.Sqrt, bias=eps_t)
                vv = small.tile([P, NT], F32, tag="vv", bufs=4)
                nc.vector.reciprocal_approx_fast(out=vv, in_=rsq)
                ob = outp.tile([P, NT, S], BF16, tag="ob")
                for tt in range(NT):
                    nc.gpsimd.tensor_scalar_mul(
                        ob[:, tt, :], w_sb[:, tt, :], vv[:, tt : tt + 1]
                    )
                nc.scalar.dma_start(out=out.ap()[b], in_=ob)

            def dma_x(b):
                t1 = xin.tile([P, 2 * KT1, S], F8, tag="x1", bufs=3)
                nc.sync.dma_start(out=t1, in_=x1hl.ap()[b])
                t2 = xin.tile([P, 2 * KT2, S], F8, tag="x2", bufs=3)
                nc.sync.dma_start(out=t2, in_=x2hl.ap()[b])
                tb = xin.tile([P, KT1, S], BF16, tag="x1b", bufs=3)
                nc.gpsimd.dma_start(out=tb, in_=x1b.ap()[b])
                return t1, t2, tb

            pending = None  # (b, x1b_sb, expz) awaiting stages B+C
            fins = []  # (b, w_sb, sumsq) awaiting finalize, flushed in pairs
            x1_cur, x2_cur, x1b_cur = x1_first, x2_first, x1b_first
            for i, b in enumerate(batches):
                if i + 1 < len(batches):
                    nxt = dma_x(batches[i + 1])
                else:
                    nxt = (None, None, None)
                prev = pending
                hold = {}

                def mid_cb():
                    hold["rbc"] = stage_b(prev[0], prev[2])

                expz = stage_a(
                    b, x1_cur, x2_cur, mid_cb if prev is not None else None
                )
                if prev is not None:
                    fins.append(
                        (prev[0],) + stage_c(prev[0], prev[1], prev[2], hold["rbc"])
                    )
                    if len(fins) == 2:
                        for f in fins:
                            stage_fin(*f)
                        fins = []
                pending = (b, x1b_cur, expz)
                x1_cur, x2_cur, x1b_cur = nxt
            # drain: flush ready finalizes BEFORE the last batch's stage B/C
            # so only the final batch's epilogue is exposed in the tail
            rbc_last = stage_b(pending[0], pending[2])
            for f in fins:
                stage_fin(*f)
            fins = [
                (pending[0],) + stage_c(pending[0], pending[1], pending[2], rbc_last)
            ]
            for f in fins:
                stage_fin(*f)

    nc.compile()
    return nc


def _split_fp8(a):
    """Split float32 array into hi+lo float8_e4m3 digits."""
    hi = a.astype(E4NP)
    lo = (a - hi.astype(np.float32)).astype(E4NP)
    return hi, lo


def _pack_pmajor(a, nchunks):
    """[.., nchunks*P, S] -> [.., P, nchunks, S] partition-major contiguous."""
    lead = a.shape[:-2]
    a = a.reshape(lead + (nchunks, P, S))
    perm = tuple(range(len(lead))) + (len(lead) + 1, len(lead), len(lead) + 2)
    return np.ascontiguousarray(a.transpose(perm))


_nc_cache = None


def kernel(x1, x2, W_query, W_key, W_value, _trace=False):
    global _nc_cache, last_results
    x1t = np.asarray(x1, dtype=np.float32).transpose(0, 2, 1)  # [B, t, s]
    x2t = np.asarray(x2, dtype=np.float32).transpose(0, 2, 1)  # [B, f, s]

    x1h, x1l = _split_fp8(x1t)
    x1hl = _pack_pmajor(np.concatenate([x1h, x1l], axis=1), 2 * KT1)
    x2h, x2l = _split_fp8(x2t)
    x2hl = _pack_pmajor(np.concatenate([x2h, x2l], axis=1), 2 * KT2)
    x1b = _pack_pmajor(x1t.astype(BFNP), KT1)

    wqh, wql = _split_fp8(np.asarray(W_query, dtype=np.float32) * WSCALE)
    wq = _pack_pmajor(np.concatenate([wqh, wql], axis=0), 2 * KT1)
    wkh, wkl = _split_fp8(np.asarray(W_key, dtype=np.float32) * WSCALE)
    wk = _pack_pmajor(np.concatenate([wkh, wkl], axis=0), 2 * KT2)
    wv = _pack_pmajor(np.asarray(W_value, dtype=np.float32).astype(BFNP), NT)

    if _nc_cache is None:
        _nc_cache = build_nc()
    nc = _nc_cache

    in_maps = []
    for c in range(N_CORES):
        sl = slice(c * BPC, (c + 1) * BPC)
        in_maps.append(
            {
                "x1hl": x1hl[sl],
                "x2hl": x2hl[sl],
                "x1b": x1b[sl],
                "wq": wq,
                "wk": wk,
                "wv": wv,
            }
        )
    res = run_bass_kernel_spmd(
        nc, in_maps, core_ids=list(range(N_CORES)), trace=_trace
    )
    last_results = res
    # out: [bpc, P, NT, S] bf16 -> outT [B, S, S] -> untranspose
    outs = [np.asarray(res.results[c]["out"]) for c in range(N_CORES)]
    outT = np.concatenate(outs, axis=0).astype(np.float32)
    outT = outT.transpose(0, 2, 1, 3).reshape(B, S, S)
    return np.ascontiguousarray(outT.transpose(0, 2, 1))


# revision 3
# speedup vs baseline: 2.5232x; 2.5232x over previous
"""Trainium2 Bass kernel for nn_AttentionLayer (B=64, S=512, F=256), 8 cores.

Reference computation (per batch b):
    scores = x1 @ Wq + x2 @ Wk          # [S, S]
    a = softmax(tanh(scores), axis=-1)   # softmax over u
    a2 = a @ Wv                          # [S, S]
    out = a2 * x1                        # elementwise
    out = out * rsqrt(max(sum_s out^2, eps))   # l2-normalize over axis s

Strategy: pure data parallelism -- 8 batches per core, weights replicated.
Everything is computed in a TRANSPOSED layout ([t-or-u partitions, s free]):
the host feeds x1^T and x2^T so both matmul stages consume operands with the
contraction dim on partitions; the softmax denominator comes from a
ones-vector matmul over partitions.

v3 design notes (informed by HW traces):
  * Stage-A matmuls in float32r (1 cycle/row at moving>=256 -- same PE rate
    as bf16; fp8 DoubleRow is only 2x per MAC on HW, so a hi+lo split-fp8
    costs 1.5x f32r and loses).  Stage C in bf16 on the UNNORMALIZED expz;
    the softmax 1/denominator is folded into the epilogue so no PE work
    waits on the rowsum->recip->broadcast chain.
  * Epilogue is dtype-pure f32 on DVE (mixed-dtype DVE ops hit a ~2.6x
    slow path on HW): q = y*x1 and w = q*recip_bc as plain tensor_tensor,
    sum-of-squares on ACT (Square + free-axis accumulator; Square shares
    the exp/tanh activation table so no table swap), final per-row
    1/sqrt scale on GpSimd normalize_recip (native Q7 op; gpsimd
    tensor_scalar is a software fallback at ~7.6us/tile -- never use).
  * The softmax reciprocal is broadcast in f32 (no bf16 cast op).
  * All DRAM tensors are packed partition-major on the host so each batch
    needs one DMA call per tensor with >=2KB contiguous lines; output is
    stored bf16 and upcast/untransposed on the host.
  * The batch loop is software-pipelined with a 1-batch skew; the previous
    batch's rowsum matmuls are emitted between the two stage-A halves.
"""

import sys

sys.path.insert(0, "/opt/trn_rl_repo")

import numpy as np
import ml_dtypes

import concourse.bass as bass
import concourse.tile as tile
from concourse import bacc, mybir
from concourse.bass_utils import run_bass_kernel_spmd

B, S, F = 64, 512, 256
N_CORES = 8
BPC = B // N_CORES  # batches per core
P = 128
KT1 = S // P  # 4 k-tiles over t (x1/Wq contraction)
KT2 = F // P  # 2 k-tiles over f (x2/Wk contraction)
NT = S // P  # 4 m-tiles over u (stage A) / t (stage C)
EPS = 1e-12

F32 = mybir.dt.float32
F32R = mybir.dt.float32r
BF16 = mybir.dt.bfloat16
AF = mybir.ActivationFunctionType
ALU = mybir.AluOpType

BFNP = ml_dtypes.bfloat16

last_results = None  # test harness introspection


def build_nc(reps=1, bpc=BPC):
    nc = bacc.Bacc(
        "TRN2", target_bir_lowering=False, debug=False, num_devices=N_CORES
    )
    # Partition-major packed tensors: [.., P, ktiles, S] -> one DMA per use.
    x1t = nc.declare_dram_parameter("x1t", [bpc, P, KT1, S], F32R, isOutput=False)
    x2t = nc.declare_dram_parameter("x2t", [bpc, P, KT2, S], F32R, isOutput=False)
    wq = nc.declare_dram_parameter("wq", [P, KT1, S], F32R, isOutput=False)
    wk = nc.declare_dram_parameter("wk", [P, KT2, S], F32R, isOutput=False)
    wv = nc.declare_dram_parameter("wv", [P, NT, S], BF16, isOutput=False)
    out = nc.declare_dram_parameter("out", [bpc, P, NT, S], BF16, isOutput=True)

    batches = [bb for _ in range(reps) for bb in range(bpc)]

    with tile.TileContext(nc) as tc:
        with (
            tc.tile_pool(name="singles", bufs=1) as singles,
            tc.tile_pool(name="xin", bufs=1) as xin,
            tc.tile_pool(name="work", bufs=2) as work,
            tc.tile_pool(name="small", bufs=2) as small,
            tc.tile_pool(name="outp", bufs=2) as outp,
            tc.tile_pool(name="psA", bufs=2, space="PSUM") as psA,
            tc.tile_pool(name="psY", bufs=3, space="PSUM") as psY,
            tc.tile_pool(name="psR", bufs=1, space="PSUM") as psR,
        ):
            # Startup: first batch's x1 k-pair 0 lands first so the first
            # matmul can start after ~0.5MB; weights round-robin across
            # queues to avoid serializing on one.
            b0 = batches[0]
            engs = [nc.scalar, nc.gpsimd, nc.scalar, nc.gpsimd]
            x1_first = xin.tile([P, KT1, S], F32R, tag="x1", bufs=3)
            wq_t = singles.tile([P, KT1, S], F32R, tag="wq")
            nc.sync.dma_start(out=x1_first[:, 0:2, :], in_=x1t.ap()[b0, :, 0:2, :])
            for c in range(2):
                engs[c].dma_start(
                    out=wq_t[:, 2 * c : 2 * c + 2, :],
                    in_=wq.ap()[:, 2 * c : 2 * c + 2, :],
                )
            nc.sync.dma_start(out=x1_first[:, 2:4, :], in_=x1t.ap()[b0, :, 2:4, :])
            wk_t = singles.tile([P, KT2, S], F32R, tag="wk")
            nc.scalar.dma_start(out=wk_t, in_=wk.ap())
            x2_first = xin.tile([P, KT2, S], F32R, tag="x2", bufs=3)
            nc.sync.dma_start(out=x2_first, in_=x2t.ap()[b0])
            wv_t = singles.tile([P, NT, S], BF16, tag="wv")
            nc.gpsimd.dma_start(out=wv_t, in_=wv.ap())

            ones_col = singles.tile([P, 1], BF16)
            nc.vector.memset(ones_col, 1.0)
            eps_t = singles.tile([P, 1], F32)
            nc.vector.memset(eps_t, EPS)

            def stage_a(b, x1_sb, x2_sb, mid_cb=None):
                """scores matmuls (f32r) in u-tile pairs sharing one 2-bank
                PSUM tile, tanh+exp over pairs.  mid_cb (if set) is emitted
                between the two pair-halves so the previous batch's rowsum
                overlaps this batch's remaining matmuls."""
                expz = work.tile([P, NT, S], BF16, tag="expz", bufs=3)
                for half in range(NT // 2):
                    sc = psA.tile([P, 2, S], F32, tag="scores")
                    for j in range(2):
                        ut = half * 2 + j
                        us = slice(ut * P, (ut + 1) * P)
                        for kt in range(KT1):
                            nc.tensor.matmul(
                                sc[:, j, :],
                                wq_t[:, kt, us],
                                x1_sb[:, kt, :],
                                start=(kt == 0),
                                stop=False,
                            )
                        for kt in range(KT2):
                            nc.tensor.matmul(
                                sc[:, j, :],
                                wk_t[:, kt, us],
                                x2_sb[:, kt, :],
                                start=False,
                                stop=(kt == KT2 - 1),
                            )
                    tanh_t = work.tile([P, 2, S], F32, tag="tanh")
                    nc.scalar.activation(out=tanh_t, in_=sc, func=AF.Tanh)
                    nc.scalar.activation(
                        out=expz[:, half * 2 : half * 2 + 2, :],
                        in_=tanh_t,
                        func=AF.Exp,
                    )
                    if half == 0 and mid_cb is not None:
                        mid_cb()
                return expz

            def stage_b(b, expz):
                """softmax denominator: rowsum matmul -> recip -> f32 bcast."""
                rs = psR.tile([1, S], F32, tag="rowsum")
                for ut in range(NT):
                    nc.tensor.matmul(
                        rs,
                        ones_col,
                        expz[:, ut, :],
                        start=(ut == 0),
                        stop=(ut == NT - 1),
                    )
                recip_f = small.tile([1, S], F32, tag="recipf")
                nc.vector.reciprocal_approx_fast(out=recip_f, in_=rs)
                rbc = small.tile([P, S], F32, tag="rbc")
                nc.gpsimd.partition_broadcast(rbc, recip_f)
                return rbc

            def stage_c(b, x1_sb, expz, rbc):
                """Y matmuls on raw expz; epilogue q=y*x1 -> w=q*rbc (f32 on
                DVE), sum-of-squares on ACT Square+accumulate."""
                w_sb = outp.tile([P, NT, S], F32, tag="w", bufs=3)
                sumsq = small.tile([P, NT], F32, tag="sumsq", bufs=4)
                for tt in range(NT):
                    y = psY.tile([P, S], F32, tag="y")
                    for ut in range(NT):
                        nc.tensor.matmul(
                            y,
                            wv_t[:, ut, tt * P : (tt + 1) * P],
                            expz[:, ut, :],
                            start=(ut == 0),
                            stop=(ut == NT - 1),
                        )
                    q_t = small.tile([P, S], F32, tag="q")
                    nc.vector.tensor_tensor(
                        out=q_t, in0=y, in1=x1_sb[:, tt, :].bitcast(F32), op=ALU.mult
                    )
                    w_t = w_sb[:, tt, :]
                    nc.vector.tensor_tensor(out=w_t, in0=q_t, in1=rbc, op=ALU.mult)
                    scr = small.tile([P, S], BF16, tag="scr")
                    nc.scalar.activation(
                        out=scr,
                        in_=w_t,
                        func=AF.Square,
                        accum_out=sumsq[:, tt : tt + 1],
                    )
                return w_sb, sumsq

            def stage_fin(b, w_sb, sumsq):
                """sqrt (ACT, emitted adjacently for pairs of batches to halve
                activation-table swaps), GpSimd normalize, store bf16."""
                rsq = small.tile([P, NT], F32, tag="rsq", bufs=4)
                nc.scalar.activation(out=rsq, in_=sumsq, func=AF.Sqrt, bias=eps_t)
                ob = outp.tile([P, NT, S], BF16, tag="ob")
                for tt in range(NT):
                    nc.gpsimd.normalize_recip(
                        out_ap=ob[:, tt, :],
                        in_ap=w_sb[:, tt, :],
                        denom_ap=rsq[:, tt : tt + 1],
                    )
                nc.scalar.dma_start(out=out.ap()[b], in_=ob)

            def dma_x(b):
                t1 = xin.tile([P, KT1, S], F32R, tag="x1", bufs=3)
                nc.sync.dma_start(out=t1[:, 0:2, :], in_=x1t.ap()[b, :, 0:2, :])
                nc.sync.dma_start(out=t1[:, 2:4, :], in_=x1t.ap()[b, :, 2:4, :])
                t2 = xin.tile([P, KT2, S], F32R, tag="x2", bufs=3)
                nc.sync.dma_start(out=t2, in_=x2t.ap()[b])
                return t1, t2

            pending = None  # (b, x1_sb, expz) awaiting stages B+C
            fins = []  # (b, w_sb, sumsq) awaiting finalize, flushed in pairs
            x1_cur, x2_cur = x1_first, x2_first
            for i, b in enumerate(batches):
                if i + 1 < len(batches):
                    nxt = dma_x(batches[i + 1])
                else:
                    nxt = (None, None)
                prev = pending
                hold = {}

                def mid_cb():
                    hold["rbc"] = stage_b(prev[0], prev[2])

                expz = stage_a(
                    b, x1_cur, x2_cur, mid_cb if prev is not None else None
                )
                if prev is not None:
                    fins.append(
                        (prev[0],) + stage_c(prev[0], prev[1], prev[2], hold["rbc"])
                    )
                    if len(fins) == 2:
                        for f in fins:
                            stage_fin(*f)
                        fins = []
                pending = (b, x1_cur, expz)
                x1_cur, x2_cur = nxt
            # drain: flush any ready finalizes BEFORE the last batch's stage
            # B/C so only the final batch's epilogue is exposed in the tail
            rbc_last = stage_b(pending[0], pending[2])
            for f in fins:
                stage_fin(*f)
            fins = [
                (pending[0],) + stage_c(pending[0], pending[1], pending[2], rbc_last)
            ]
            for f in fins:
                stage_fin(*f)

    nc.compile()
    return nc


def _pack_pmajor(a, nchunks):
    """[.., nchunks*P, S] -> [.., P, nchunks, S] partition-major contiguous."""
    lead = a.shape[:-2]
    a = a.reshape(lead + (nchunks, P, S))
    perm = tuple(range(len(lead))) + (len(lead) + 1, len(lead), len(lead) + 2)
    return np.ascontiguousarray(a.transpose(perm))


_nc_cache = None


def kernel(x1, x2, W_query, W_key, W_value, _trace=False):
    global _nc_cache, last_results
    x1t = _pack_pmajor(
        np.asarray(x1, dtype=np.float32).transpose(0, 2, 1), KT1
    )  # [B, P, KT1, S]
    x2t = _pack_pmajor(np.asarray(x2, dtype=np.float32).transpose(0, 2, 1), KT2)
    wq = _pack_pmajor(np.asarray(W_query, dtype=np.float32), KT1)
    wk = _pack_pmajor(np.asarray(W_key, dtype=np.float32), KT2)
    wv = _pack_pmajor(np.asarray(W_value, dtype=np.float32).astype(BFNP), NT)

    if _nc_cache is None:
        _nc_cache = build_nc()
    nc = _nc_cache

    in_maps = []
    for c in range(N_CORES):
        sl = slice(c * BPC, (c + 1) * BPC)
        in_maps.append(
            {"x1t": x1t[sl], "x2t": x2t[sl], "wq": wq, "wk": wk, "wv": wv}
        )
    res = run_bass_kernel_spmd(
        nc, in_maps, core_ids=list(range(N_CORES)), trace=_trace
    )
    last_results = res
    # out: [bpc, P, NT, S] bf16 -> outT [B, S, S] -> untranspose
    outs = [np.asarray(res.results[c]["out"]) for c in range(N_CORES)]
    outT = np.concatenate(outs, axis=0).astype(np.float32)
    outT = outT.transpose(0, 2, 1, 3).reshape(B, S, S)
    return np.ascontiguousarray(outT.transpose(0, 2, 1))


# revision 6
# speedup vs baseline: 2.8164x; 1.1162x over previous
"""Trainium2 Bass kernel for nn_AttentionLayer (B=64, S=512, F=256), 8 cores.

Reference computation (per batch b):
    scores = x1 @ Wq + x2 @ Wk          # [S, S]
    a = softmax(tanh(scores), axis=-1)   # softmax over u
    a2 = a @ Wv                          # [S, S]
    out = a2 * x1                        # elementwise
    out = out * rsqrt(max(sum_s out^2, eps))   # l2-normalize over axis s

Strategy: pure data parallelism -- 8 batches per core, weights replicated.
Everything is computed in a TRANSPOSED layout ([t-or-u partitions, s free]).

v4 design notes (informed by HW traces):
  * Stage-A x1 matmuls in float32r (same PE rate as bf16 on HW; fp8
    DoubleRow is only 2x per MAC so any accuracy-safe hi+lo split loses).
    The x2@Wk branch and stage C run in bf16.  Stage C consumes the
    UNNORMALIZED expz; 1/denominator is folded into the epilogue, so no PE
    work waits on the rowsum->recip->broadcast chain.
  * Epilogue is dtype-pure f32 on DVE (mixed-dtype DVE ops hit a ~2.6x
    slow path): q = y*x1, w = q*recip_bc as tensor_tensor; sum-of-squares
    split between ACT Square+accum (shares the exp/tanh table -- no swap)
    and DVE stt to balance engine load; final per-row 1/sqrt scale on
    GpSimd normalize_recip (native; gpsimd tensor_scalar is a ~7.6us/tile
    software trap -- never use).
  * Startup: weights + first batch issued per-ktile in exact consumption
    order across four DMA queues (sync/scalar/gpsimd/vector).
  * Drain: the last batch's squares run on DVE so the tail has exactly one
    activation-table load (Sqrt x2), its normalizes are split Pool/DVE,
    and its output is stored in two half-DMAs.
  * Output bf16, upcast + untransposed on host.
"""

import sys

sys.path.insert(0, "/opt/trn_rl_repo")

import numpy as np
import ml_dtypes

import concourse.bass as bass
import concourse.tile as tile
from concourse import bacc, mybir
from concourse.bass_utils import run_bass_kernel_spmd

B, S, F = 64, 512, 256
N_CORES = 8
BPC = B // N_CORES  # batches per core
P = 128
KT1 = S // P  # 4 k-tiles over t (x1/Wq contraction)
KT2 = F // P  # 2 k-tiles over f (x2/Wk contraction)
NT = S // P  # 4 m-tiles over u (stage A) / t (stage C)
EPS = 1e-12

F32 = mybir.dt.float32
F32R = mybir.dt.float32r
BF16 = mybir.dt.bfloat16
AF = mybir.ActivationFunctionType
ALU = mybir.AluOpType

BFNP = ml_dtypes.bfloat16

last_results = None  # test harness introspection


def build_nc(reps=1, bpc=BPC):
    nc = bacc.Bacc(
        "TRN2", target_bir_lowering=False, debug=False, num_devices=N_CORES
    )
    # Partition-major packed tensors: [.., P, ktiles, S].
    x1t = nc.declare_dram_parameter("x1t", [bpc, P, KT1, S], F32R, isOutput=False)
    x2t = nc.declare_dram_parameter("x2t", [bpc, P, KT2, S], BF16, isOutput=False)
    wq = nc.declare_dram_parameter("wq", [P, KT1, S], F32R, isOutput=False)
    wk = nc.declare_dram_parameter("wk", [P, KT2, S], BF16, isOutput=False)
    wv = nc.declare_dram_parameter("wv", [P, NT, S], BF16, isOutput=False)
    out = nc.declare_dram_parameter("out", [bpc, P, NT, S], BF16, isOutput=True)

    batches = [bb for _ in range(reps) for bb in range(bpc)]

    with tile.TileContext(nc) as tc:
        with (
            tc.tile_pool(name="singles", bufs=1) as singles,
            tc.tile_pool(name="xin", bufs=1) as xin,
            tc.tile_pool(name="work", bufs=2) as work,
            tc.tile_pool(name="small", bufs=2) as small,
            tc.tile_pool(name="outp", bufs=2) as outp,
            tc.tile_pool(name="psA", bufs=2, space="PSUM") as psA,
            tc.tile_pool(name="psY", bufs=3, space="PSUM") as psY,
            tc.tile_pool(name="psR", bufs=1, space="PSUM") as psR,
        ):
            # Startup DMAs in consumption order: the first u-tile needs
            # wq.kt + x1.kt pairs in sequence, then wk + x2.
            b0 = batches[0]
            x1_first = xin.tile([P, KT1, S], F32R, tag="x1", bufs=3)
            wq_t = singles.tile([P, KT1, S], F32R, tag="wq")
            wk_t = singles.tile([P, KT2, S], BF16, tag="wk")
            x2_first = xin.tile([P, KT2, S], BF16, tag="x2", bufs=3)
            wv_t = singles.tile([P, NT, S], BF16, tag="wv")
            wq_engs = [nc.scalar, nc.scalar, nc.gpsimd, nc.gpsimd]
            for kt in range(KT1):
                wq_engs[kt].dma_start(
                    out=wq_t[:, kt : kt + 1, :], in_=wq.ap()[:, kt : kt + 1, :]
                )
                nc.sync.dma_start(
                    out=x1_first[:, kt : kt + 1, :],
                    in_=x1t.ap()[b0, :, kt : kt + 1, :],
                )
            nc.scalar.dma_start(out=wk_t, in_=wk.ap())
            nc.sync.dma_start(out=x2_first, in_=x2t.ap()[b0])
            nc.gpsimd.dma_start(out=wv_t, in_=wv.ap())

            ones_col = singles.tile([P, 1], BF16)
            nc.vector.memset(ones_col, 1.0)
            eps_t = singles.tile([P, 1], F32)
            nc.vector.memset(eps_t, EPS)

            def stage_a(b, x1_sb, x2_sb, mid_cb=None):
                """scores matmuls in u-tile pairs sharing one 2-bank PSUM
                tile, tanh+exp over pairs.  mid_cb (if set) is emitted
                between the two pair-halves so the previous batch's rowsum
                overlaps this batch's remaining matmuls."""
                expz = work.tile([P, NT, S], BF16, tag="expz", bufs=3)
                for half in range(NT // 2):
                    sc = psA.tile([P, 2, S], F32, tag="scores")
                    for j in range(2):
                        ut = half * 2 + j
                        us = slice(ut * P, (ut + 1) * P)
                        for kt in range(KT1):
                            nc.tensor.matmul(
                                sc[:, j, :],
                                wq_t[:, kt, us],
                                x1_sb[:, kt, :],
                                start=(kt == 0),
                                stop=False,
                            )
                        for kt in range(KT2):
                            nc.tensor.matmul(
                                sc[:, j, :],
                                wk_t[:, kt, us],
                                x2_sb[:, kt, :],
                                start=False,
                                stop=(kt == KT2 - 1),
                            )
                    tanh_t = work.tile([P, 2, S], F32, tag="tanh")
                    nc.scalar.activation(out=tanh_t, in_=sc, func=AF.Tanh)
                    nc.scalar.activation(
                        out=expz[:, half * 2 : half * 2 + 2, :],
                        in_=tanh_t,
                        func=AF.Exp,
                    )
                    if half == 0 and mid_cb is not None:
                        mid_cb()
                return expz

            def stage_b(b, expz):
                """softmax denominator: rowsum matmul -> recip -> f32 bcast."""
                rs = psR.tile([1, S], F32, tag="rowsum")
                for ut in range(NT):
                    nc.tensor.matmul(
                        rs,
                        ones_col,
                        expz[:, ut, :],
                        start=(ut == 0),
                        stop=(ut == NT - 1),
                    )
                recip_f = small.tile([1, S], F32, tag="recipf")
                nc.vector.reciprocal_approx_fast(out=recip_f, in_=rs)
                rbc = small.tile([P, S], F32, tag="rbc")
                nc.gpsimd.partition_broadcast(rbc, recip_f)
                return rbc

            def stage_c(b, x1_sb, expz, rbc, sq_on_dve=False):
                """Y matmuls on raw expz; epilogue q=y*x1 -> w=q*rbc (f32 on
                DVE); sum-of-squares split ACT/DVE (all-DVE for the drain
                batch so the tail needs no extra activation-table swap)."""
                w_sb = outp.tile([P, NT, S], F32, tag="w", bufs=3)
                sumsq = small.tile([P, NT], F32, tag="sumsq", bufs=4)
                for tt in range(NT):
                    y = psY.tile([P, S], F32, tag="y")
                    for ut in range(NT):
                        nc.tensor.matmul(
                            y,
                            wv_t[:, ut, tt * P : (tt + 1) * P],
                            expz[:, ut, :],
                            start=(ut == 0),
                            stop=(ut == NT - 1),
                        )
                    q_t = small.tile([P, S], F32, tag="q")
                    nc.vector.tensor_tensor(
                        out=q_t, in0=y, in1=x1_sb[:, tt, :].bitcast(F32), op=ALU.mult
                    )
                    w_t = w_sb[:, tt, :]
                    nc.vector.tensor_tensor(out=w_t, in0=q_t, in1=rbc, op=ALU.mult)
                    if sq_on_dve or tt >= 2:
                        scr = small.tile([P, S], F32, tag="scr")
                        nc.vector.scalar_tensor_tensor(
                            out=scr,
                            in0=w_t,
                            scalar=1.0,
                            in1=w_t,
                            op0=ALU.mult,
                            op1=ALU.mult,
                            accum_out=sumsq[:, tt : tt + 1],
                        )
                    else:
                        scr = small.tile([P, S], BF16, tag="scrb")
                        nc.scalar.activation(
                            out=scr,
                            in_=w_t,
                            func=AF.Square,
                            accum_out=sumsq[:, tt : tt + 1],
                        )
                return w_sb, sumsq

            def stage_fin(b, w_sb, sumsq):
                """sqrt (ACT, emitted adjacently for pairs of batches to halve
                activation-table swaps), GpSimd normalize, store bf16."""
                rsq = small.tile([P, NT], F32, tag="rsq", bufs=4)
                nc.scalar.activation(out=rsq, in_=sumsq, func=AF.Sqrt, bias=eps_t)
                ob = outp.tile([P, NT, S], BF16, tag="ob")
                for tt in range(NT):
                    nc.gpsimd.normalize_recip(
                        out_ap=ob[:, tt, :],
                        in_ap=w_sb[:, tt, :],
                        denom_ap=rsq[:, tt : tt + 1],
                    )
                nc.scalar.dma_start(out=out.ap()[b], in_=ob)

            def stage_fin_last(b, w_sb, sumsq):
                """Drain-batch finalize: normalizes split Pool/DVE, output
                stored in two half-DMAs so only ~0.25MB is tail-exposed."""
                rsq = small.tile([P, NT], F32, tag="rsq", bufs=4)
                nc.scalar.activation(out=rsq, in_=sumsq, func=AF.Sqrt, bias=eps_t)
                vv = small.tile([P, NT], F32, tag="vv")
                nc.vector.reciprocal_approx_fast(out=vv, in_=rsq)
                ob = outp.tile([P, NT, S], BF16, tag="ob")
                for tt in range(NT):
                    if tt % 2 == 0:
                        nc.gpsimd.normalize_recip(
                            out_ap=ob[:, tt, :],
                            in_ap=w_sb[:, tt, :],
                            denom_ap=rsq[:, tt : tt + 1],
                        )
                    else:
                        nc.vector.tensor_scalar_mul(
                            ob[:, tt, :], w_sb[:, tt, :], vv[:, tt : tt + 1]
                        )
                    if tt == 1:
                        nc.scalar.dma_start(
                            out=out.ap()[b, :, 0:2, :], in_=ob[:, 0:2, :]
                        )
                nc.scalar.dma_start(out=out.ap()[b, :, 2:4, :], in_=ob[:, 2:4, :])

            def dma_x(b):
                t1 = xin.tile([P, KT1, S], F32R, tag="x1", bufs=3)
                nc.sync.dma_start(out=t1[:, 0:2, :], in_=x1t.ap()[b, :, 0:2, :])
                nc.sync.dma_start(out=t1[:, 2:4, :], in_=x1t.ap()[b, :, 2:4, :])
                t2 = xin.tile([P, KT2, S], BF16, tag="x2", bufs=3)
                nc.sync.dma_start(out=t2, in_=x2t.ap()[b])
                return t1, t2

            pending = None  # (b, x1_sb, expz) awaiting stages B+C
            fins = []  # (b, w_sb, sumsq) awaiting finalize, flushed in pairs
            x1_cur, x2_cur = x1_first, x2_first
            for i, b in enumerate(batches):
                if i + 1 < len(batches):
                    nxt = dma_x(batches[i + 1])
                else:
                    nxt = (None, None)
                prev = pending
                hold = {}

                def mid_cb():
                    hold["rbc"] = stage_b(prev[0], prev[2])

                expz = stage_a(
                    b, x1_cur, x2_cur, mid_cb if prev is not None else None
                )
                if prev is not None:
                    fins.append(
                        (prev[0],) + stage_c(prev[0], prev[1], prev[2], hold["rbc"])
                    )
                    if len(fins) == 2:
                        for f in fins:
                            stage_fin(*f)
                        fins = []
                pending = (b, x1_cur, expz)
                x1_cur, x2_cur = nxt
            # drain: penultimate finalize overlaps the last batch's stage C;
            # the last batch's squares run on DVE so ACT does a single
            # table-swap (Sqrt for both final batches, back to back).
            rbc_last = stage_b(pending[0], pending[2])
            last_c = stage_c(
                pending[0], pending[1], pending[2], rbc_last, sq_on_dve=True
            )
            for f in fins:
                stage_fin(*f)
            stage_fin_last(pending[0], *last_c)

    nc.compile()
    return nc


def _pack_pmajor(a, nchunks):
    """[.., nchunks*P, S] -> [.., P, nchunks, S] partition-major contiguous."""
    lead = a.shape[:-2]
    a = a.reshape(lead + (nchunks, P, S))
    perm = tuple(range(len(lead))) + (len(lead) + 1, len(lead), len(lead) + 2)
    return np.ascontiguousarray(a.transpose(perm))


_nc_cache = None


def kernel(x1, x2, W_query, W_key, W_value, _trace=False):
    global _nc_cache, last_results
    x1t = _pack_pmajor(
        np.asarray(x1, dtype=np.float32).transpose(0, 2, 1), KT1
    )  # [B, P, KT1, S]
    x2t = _pack_pmajor(
        np.asarray(x2, dtype=np.float32).transpose(0, 2, 1).astype(BFNP), KT2
    )
    wq = _pack_pmajor(np.asarray(W_query, dtype=np.float32), KT1)
    wk = _pack_pmajor(np.asarray(W_key, dtype=np.float32).astype(BFNP), KT2)
    wv = _pack_pmajor(np.asarray(W_value, dtype=np.float32).astype(BFNP), NT)

    if _nc_cache is None:
        _nc_cache = build_nc()
    nc = _nc_cache

    in_maps = []
    for c in range(N_CORES):
        sl = slice(c * BPC, (c + 1) * BPC)
        in_maps.append(
            {"x1t": x1t[sl], "x2t": x2t[sl], "wq": wq, "wk": wk, "wv": wv}
        )
    res = run_bass_kernel_spmd(
        nc, in_maps, core_ids=list(range(N_CORES)), trace=_trace
    )
    last_results = res
    # out: [bpc, P, NT, S] bf16 -> outT [B, S, S] -> untranspose
    outs = [np.asarray(res.results[c]["out"]) for c in range(N_CORES)]
    outT = np.concatenate(outs, axis=0).astype(np.float32)
    outT = outT.transpose(0, 2, 1, 3).reshape(B, S, S)
    return np.ascontiguousarray(outT.transpose(0, 2, 1))
